# revision 70
# baseline (speedup 1.0000x reference)
"""TRN2 Bass kernel for soft 2D polygon rasterization (1024x1024, 64-edge star).

Architecture (one SPMD program on 8 cores, per-core behavior data-driven):
  - Layout: y (rows) on partitions (local row within a 128-row octant), x
    (columns) on the free axis. 64 tiles of [128 rows x 128 cols]; each core
    processes 8, assigned by a pad-aware host-side load balancer.
  - Parity: signed crossing histogram per column (alternating +-1 in sorted
    order -> prefix sum is parity 0/1); the (base - 0.5) octant offset is
    folded into histogram row 0, so par' = parity - 0.5 = +-0.5 exactly.
    All values are bf16-exact, so parity is a few bf16 matmuls
    (U-triangular stationary, hist streamed), parked in PSUM.
    sd2 = par' * d2 (one TT mult per 2 phases), val = sigmoid(2*sd2).
  - Distance d2min per pixel = min over per-tile candidate surfaces within
    R_KEEP = 2.2 px reach (missed-feature err <= sigmoid(-R^2) ~ 8e-3):
    edge cand = max(w, c^2) with w = K2*(v_tangent^2 - (L/2)^2) a coarse
    overshoot test (vertex discs cover the +-DELTA endpoint band exactly);
    vertex cand = endpoint disc distance^2.  Host-side ray/wedge tests
    drop w-tests and vertex discs that cannot affect the tile; padded
    dummy slots carry real cheap edges instead of constants.
  - ALL candidate surfaces (w, c^2, discs) are quadratics in (x, y) and are
    computed on the TensorEngine as ONE K=12 bf16 matmul each per 128-col
    block: basis rows (1, y', y2hi, y2lo) x 3, with every quad coefficient
    triple-split into bf16 pieces (bf16 x bf16 products are exact in the
    fp32 PSUM accumulator, so the result carries ~2^-24 relative accuracy
    at 1 cycle/column -- 8x cheaper than fp32's two-pass matmul).
    Per-tile recentered coordinates keep term magnitudes ~1e4.
  - Per phase: PSUM subtiles of <=12 blocks are copied (wide ACT/DVE ops)
    into a bf16 work tile; one scalar_tensor_tensor folds max(w, c^2)
    reading the w blocks straight from PSUM; a block-halving bf16 TT-min
    tree folds the T candidates to d2; sigmoid saturation handles the
    far field exactly.
  - The bbox band test and out-of-band zeroing are done by the host during
    output assembly (pure row/col masks).
"""
import os
import numpy as np

W = H = 1024
NCORES = 8
OCT_H = 128          # tile rows
NOCT = 8             # tiles per core
THRESHOLD = 30.0
R_KEEP = 2.2         # cull radius (missed-feature err <= sigmoid(-R^2) ~ 8e-3)
W_TARGET = 40.0      # w overshoot test must exceed this at overshoot >= DELTA
DELTA = 0.15         # vertex disc covers |overshoot| <= DELTA exactly
DUMMY = 3600.0       # candidate value for padded slots

LAST_RESULTS = None  # BassKernelResults of the most recent run (for harness)


# ---------------------------------------------------------------------------
# host-side geometry prep
# ---------------------------------------------------------------------------

def _host_prep(polygon):
    poly = np.asarray(polygon, dtype=np.float32)
    E = poly.shape[0]
    a = poly
    b = np.roll(poly, -1, axis=0)
    ab = b - a

    # bbox band (exact f32 replication of the reference; applied on host)
    x_lo = np.float32(np.floor(poly[:, 0].min()))
    y_lo = np.float32(np.floor(poly[:, 1].min()))
    x_hi = np.float32(np.floor(poly[:, 0].max()) + np.float32(1.0))
    y_hi = np.float32(np.floor(poly[:, 1].max()) + np.float32(1.0))
    thr = np.float32(THRESHOLD)
    px = np.arange(W, dtype=np.float32)
    py = np.arange(H, dtype=np.float32)
    col_in = (px >= x_lo - thr) & (px <= x_hi + thr)
    row_in = (py >= y_lo - thr) & (py <= y_hi + thr)

    # ---- signed crossing histogram (exact f32 semantics, as baseline) ----
    PX = px[None, :]
    a0 = a[:, 0:1]; a1 = a[:, 1:2]; b0 = b[:, 0:1]
    ab0 = ab[:, 0:1]; ab1 = ab[:, 1:2]
    crosses = (a0 <= PX) != (b0 <= PX)                       # [E, W]
    safe_dx = np.where(ab0 == np.float32(0.0), np.float32(1.0), ab0)
    with np.errstate(over='ignore', invalid='ignore'):
        yint = a1 + (PX - a0) * ab1 / safe_dx                # [E, W] f32
    bins = np.where(crosses, np.ceil(yint.astype(np.float64)), np.inf)
    bins = np.where(bins < 0, 0.0, bins)
    bins = np.where(bins > H - 1, np.inf, bins)
    srt = np.sort(bins, axis=0)
    sign = np.where((np.arange(E)[:, None] % 2) == 0, 1.0, -1.0)
    hist = np.zeros((H, W), dtype=np.float64)
    valid = np.isfinite(srt)
    kk = srt[valid].astype(np.int64)
    jj = np.broadcast_to(np.arange(W)[None, :], (E, W))[valid]
    np.add.at(hist, (kk, jj), np.broadcast_to(sign, (E, W))[valid])
    csum = np.cumsum(hist, axis=0)      # parity (0/1) at row i, per column

    # ---- per-(strip, octant) culling (f64 geometry) ----
    A = a.astype(np.float64); B = b.astype(np.float64); AB = B - A
    L2 = AB[:, 0] ** 2 + AB[:, 1] ** 2
    L = np.sqrt(np.maximum(L2, 1e-12))
    good = L2 > 1e-9
    R = R_KEEP

    def _ray_rect_dist(pxx, pyy, ux, uy, rx0, rx1, ry0, ry1):
        """Min distance from sampled ray (p + t*u, t in [0,3000]) to rect."""
        ts = np.arange(0.0, 3000.0, 1.0)
        xs = pxx + ts * ux
        ys = pyy + ts * uy
        ddx = np.maximum(np.maximum(rx0 - xs, xs - rx1), 0)
        ddy = np.maximum(np.maximum(ry0 - ys, ys - ry1), 0)
        return np.sqrt(ddx * ddx + ddy * ddy).min()

    # per (strip, octant): edge entries (e, needs_w), vertex entries
    oct_edges = [[[] for _ in range(NOCT)] for _ in range(8)]
    oct_verts = [[[] for _ in range(NOCT)] for _ in range(8)]
    tn = np.stack([AB[:, 0] / L, AB[:, 1] / L], axis=1)   # unit tangents
    for s in range(8):
        xr0, xr1 = s * 128, s * 128 + 127
        for e in range(E):
            ax, ay = A[e]; bx, by = B[e]
            if good[e]:
                lo, hi = min(ax, bx), max(ax, bx)
                if not (hi < xr0 - R or lo > xr1 + R):
                    ts = [0.0, 1.0]
                    if abs(bx - ax) > 1e-12:
                        for xc in (xr0 - R, xr1 + R):
                            t = (xc - ax) / (bx - ax)
                            if 0.0 < t < 1.0:
                                ts.append(t)
                    ts = [t for t in ts
                          if xr0 - R - 1e-9 <= ax + (bx - ax) * t <= xr1 + R + 1e-9]
                    if ts:
                        ys = [ay + (by - ay) * t for t in ts]
                        ylo = max(0, int(np.floor(min(ys) - R)))
                        yhi = min(H - 1, int(np.ceil(max(ys) + R)))
                        if ylo <= yhi:
                            for o in range(NOCT):
                                yt0, yt1 = o * OCT_H, o * OCT_H + OCT_H - 1
                                if max(ylo, yt0) <= min(yhi, yt1):
                                    # does either endpoint's line-extension
                                    # danger strip reach the tile?
                                    needs_w = False
                                    for (qx, qy, sg) in ((ax, ay, -1.0),
                                                         (bx, by, 1.0)):
                                        if _ray_rect_dist(
                                                qx, qy, sg * tn[e, 0],
                                                sg * tn[e, 1], xr0, xr1,
                                                yt0, yt1) <= R + 0.9:
                                            needs_w = True
                                            break
                                    oct_edges[s][o].append((e, needs_w))
            # vertex disc at A[e]: needed only if the wedge between the
            # previous edge's extension and this edge's start reaches tile
            if xr0 - R <= ax <= xr1 + R:
                ylo = max(0, int(np.floor(ay - R)))
                yhi = min(H - 1, int(np.ceil(ay + R)))
                ep = (e - 1) % E
                tp = tn[ep]            # direction of incoming edge
                tc = tn[e]             # direction of outgoing edge
                ang = np.linspace(0, 2 * np.pi, 64, endpoint=False)
                ca, sa = np.cos(ang), np.sin(ang)
                for o in range(NOCT):
                    yt0, yt1 = o * OCT_H, o * OCT_H + OCT_H - 1
                    if max(ylo, yt0) <= min(yhi, yt1):
                        need = False
                        for r in (0.0, 0.3 * R, 0.65 * R, R):
                            qx = ax + r * ca
                            qy = ay + r * sa
                            dp = (qx - ax) * tp[0] + (qy - ay) * tp[1]
                            dc = (qx - ax) * tc[0] + (qy - ay) * tc[1]
                            wedge = (dp >= -0.35) & (dc <= 0.35)
                            intile = ((qx >= xr0 - 0.7) & (qx <= xr1 + 0.7) &
                                      (qy >= yt0 - 0.7) & (qy <= yt1 + 0.7))
                            if np.any(wedge & intile):
                                need = True
                                break
                        if need:
                            oct_verts[s][o].append(e)

    # ---- octant -> (core, phase) assignment (pad-aware local search) ----
    octs = [(s, o) for s in range(8) for o in range(NOCT)]
    nW = {so: sum(1 for _, w in oct_edges[so[0]][so[1]] if w) for so in octs}
    nC = {so: sum(1 for _, w in oct_edges[so[0]][so[1]] if not w)
          for so in octs}
    nV = {so: len(oct_verts[so[0]][so[1]]) for so in octs}
    cW, cC, cV = 2.2, 1.2, 1.0
    cost = {so: cW * nW[so] + cC * nC[so] + cV * nV[so] for so in octs}

    def padded_cost(assign):
        tot = 0.0
        ranked = [sorted(aa, key=lambda so: -cost[so]) for aa in assign]
        for k in range(NOCT):
            tot += cW * max(nW[r[k]] for r in ranked)
            tot += cC * max(nC[r[k]] for r in ranked)
            tot += cV * max(nV[r[k]] for r in ranked)
        return tot

    def class_maxima(assign):
        ranked = [sorted(aa, key=lambda so: -cost[so]) for aa in assign]
        Wk, Vk, Ck = [], [], []
        for k in range(NOCT):
            wk = max(nW[r[k]] for r in ranked)
            vk = max(nV[r[k]] for r in ranked)
            # dummy W/V slots absorb this core's cheap edges; only the
            # overflow needs dedicated cheap slots
            ck = max(max(0, nC[r[k]] - (wk - nW[r[k]]) - (vk - nV[r[k]]))
                     for r in ranked)
            Wk.append(wk); Vk.append(vk); Ck.append(ck)
        return ranked, Wk, Vk, Ck

    def padded_cost(assign):
        _, Wk, Vk, Ck = class_maxima(assign)
        return sum(cW * w + cV * v + cC * cc for w, v, cc in zip(Wk, Vk, Ck))

    order = sorted(octs, key=lambda so: -cost[so])
    core_load = [0.0] * NCORES
    assign = [[] for _ in range(NCORES)]
    for so in order:
        cands = [c for c in range(NCORES) if len(assign[c]) < NOCT]
        c = min(cands, key=lambda c: core_load[c])
        assign[c].append(so)
        core_load[c] += cost[so]
    best = padded_cost(assign)
    rng = np.random.default_rng(0)
    for _ in range(12000):
        c1, c2 = rng.integers(0, NCORES, 2)
        if c1 == c2:
            continue
        i1, i2 = rng.integers(0, NOCT, 2)
        assign[c1][i1], assign[c2][i2] = assign[c2][i2], assign[c1][i1]
        newc = padded_cost(assign)
        if newc <= best:
            best = newc
        else:
            assign[c1][i1], assign[c2][i2] = assign[c2][i2], assign[c1][i1]
    core_octs, Wk, Vk, Ck = class_maxima(assign)

    # ---- all slots are K=12 bf16 triple-split PE quads ----
    # Per phase, the quad region is [w_0..w_{W-1} | c2_0..c2_{S-1} |
    # v_0..v_{V-1}], processed in PSUM subtiles of <= QSUB blocks.  Each
    # subtile is copied to a contiguous piece of the SBUF work tile
    # [w-stage(W) | cand: c2(S), verts(V)].  One bf16 STT folds
    # max(w, c2) for the first W slots; a bf16 TT-min tree folds T->d2.
    QSUB = 12
    plan = []
    for k in range(NOCT):
        S = Wk[k] + Ck[k]
        T = S + Vk[k]
        plan.append(dict(S=S, W=Wk[k], C=Ck[k], V=Vk[k], T=T,
                         B=Wk[k] + S + Vk[k]))
    NQ = sum(p["B"] * 128 for p in plan)

    ylocal = np.arange(128, dtype=np.float64)
    yprime = ylocal - 63.5
    y2 = yprime * yprime

    import ml_dtypes

    def bfr(x):
        return np.asarray(x, dtype=np.float64).astype(
            ml_dtypes.bfloat16).astype(np.float64)

    y2h = bfr(y2)
    y2l = y2 - y2h
    basis = np.stack([np.ones(128), yprime, y2h, y2l])          # [4, 128]
    lhsT12 = np.concatenate([basis, basis, basis], axis=0)      # [12, 128]
    assert np.all(bfr(lhsT12) == lhsT12)
    xs_loc = np.arange(128, dtype=np.float64)   # x local 0..127

    in_maps = []
    for c in range(NCORES):
        qrhs = np.zeros((12, max(NQ, 1)), dtype=np.float64)
        histc = np.zeros((128, NOCT * 128), dtype=np.float64)

        def put_quad(col, q0, q1, q2):
            """Triple-split quad coeffs -> 12 bf16 rhs rows at col-block."""
            q2 = np.broadcast_to(np.asarray(q2, dtype=np.float64), (128,))
            q1 = np.broadcast_to(np.asarray(q1, dtype=np.float64), (128,))
            q0 = np.broadcast_to(np.asarray(q0, dtype=np.float64), (128,))
            r0, r1, r2 = q0, q1, q2
            for lvl in range(3):
                h0, h1, h2 = bfr(r0), bfr(r1), bfr(r2)
                qrhs[4 * lvl + 0, col:col + 128] = h0
                qrhs[4 * lvl + 1, col:col + 128] = h1
                qrhs[4 * lvl + 2, col:col + 128] = h2
                qrhs[4 * lvl + 3, col:col + 128] = h2
                r0, r1, r2 = r0 - h0, r1 - h1, r2 - h2

        qcol = 0
        for k in range(NOCT):
            p = plan[k]
            s, o = core_octs[c][k]
            i0 = o * OCT_H
            xg = s * 128 + xs_loc                 # global x per free col
            yg = i0 + ylocal                      # global y per partition
            yc = i0 + 63.5                        # tile y center
            elist = oct_edges[s][o]
            wlist = [e for e, w in elist if w]
            cheap = [e for e, w in elist if not w]
            vlist = oct_verts[s][o]

            # fill W slots: real w-edges, then cheap edges, then dummies
            wslots = [("w", e) for e in wlist]
            while len(wslots) < p["W"] and cheap:
                wslots.append(("c", cheap.pop(0)))
            while len(wslots) < p["W"]:
                wslots.append((None, None))
            # fill V slots: real verts, then cheap (as c2 quads), then dummies
            vslots = [("v", e) for e in vlist]
            while len(vslots) < p["V"] and cheap:
                vslots.append(("cq", cheap.pop(0)))
            while len(vslots) < p["V"]:
                vslots.append((None, None))
            # leftover cheap -> dedicated C slots
            cslots = [("c", e) for e in cheap]
            while len(cslots) < p["C"]:
                cslots.append((None, None))
            assert len(cslots) == p["C"], (len(cheap), p)

            def edge_c2q(e):
                """c2 quad coeffs for edge e at this tile."""
                nx, ny = AB[e, 1] / L[e], -AB[e, 0] / L[e]
                c0 = -(nx * A[e, 0] + ny * A[e, 1])
                cn = nx * xg + ny * yc + c0                        # [128] per f
                return cn * cn, 2.0 * ny * cn, ny * ny

            # --- w quads (W blocks) ---
            for si in range(p["W"]):
                kind, e = wslots[si]
                if kind == "w":
                    tx, ty = AB[e, 0] / L[e], AB[e, 1] / L[e]
                    mx, my = (A[e] + B[e]) / 2.0
                    h = L[e] / 2.0
                    K2 = W_TARGET / (max(2.0 * h, 1e-6) * DELTA)
                    v0 = tx * xg + ty * yc - (tx * mx + ty * my)   # [128] per f
                    put_quad(qcol, K2 * (v0 * v0 - h * h),
                             K2 * (2.0 * ty * v0), K2 * (ty * ty))
                else:   # cheap filler or dummy: no overshoot test
                    put_quad(qcol, -1000.0, 0.0, 0.0)
                qcol += 128
            # --- c2 quads (S blocks: W-slot edges then C-slot edges) ---
            for kind, e in wslots + cslots:
                if e is not None:
                    q0, q1, q2 = edge_c2q(e)
                    put_quad(qcol, q0, q1, q2)
                else:
                    put_quad(qcol, DUMMY, 0.0, 0.0)
                qcol += 128
            # --- vert quads (V blocks) ---
            for kind, e in vslots:
                if kind == "v":
                    axv, ayv = A[e]
                    ay_c = ayv - yc
                    dx = xg - axv                                  # [128] per f
                    put_quad(qcol, dx * dx + ay_c * ay_c, -2.0 * ay_c, 1.0)
                elif kind == "cq":   # cheap edge c2 as a quad
                    q0, q1, q2 = edge_c2q(e)
                    put_quad(qcol, q0, q1, q2)
                else:
                    put_quad(qcol, DUMMY, 0.0, 0.0)
                qcol += 128

            # --- histogram block (bf16-exact) ---
            hloc = np.array(hist[i0:i0 + OCT_H, s * 128:(s + 1) * 128])
            basep = np.mod(csum[i0 - 1, s * 128:(s + 1) * 128], 2.0) if i0 > 0 \
                else np.zeros(128)
            hloc[0, :] += basep - 0.5          # par' = parity - 0.5 = +-0.5
            histc[:, k * 128:(k + 1) * 128] = hloc

        hb = histc.astype(ml_dtypes.bfloat16)
        assert np.all(hb.astype(np.float64) == histc), "hist not bf16-exact"
        qb = qrhs.astype(ml_dtypes.bfloat16)
        assert np.all(qb.astype(np.float64) == qrhs), "qrhs not bf16-exact"
        in_maps.append({
            "hist": hb,
            "qrhs": qb,
            "lhsT12": lhsT12.astype(ml_dtypes.bfloat16),
        })
    return in_maps, core_octs, plan, NQ, row_in, col_in


# ---------------------------------------------------------------------------
# device program
# ---------------------------------------------------------------------------

def _build_program(plan, NQ):
    import concourse.bacc as bacc
    import concourse.mybir as mybir
    from concourse.tile import TileContext

    F32 = mybir.dt.float32
    BF16 = mybir.dt.bfloat16
    I32 = mybir.dt.int32
    AF = mybir.ActivationFunctionType
    OP = mybir.AluOpType

    QSUB = 12   # PSUM subtile blocks (3 banks; bufs=2 + par 2 = 8 banks)

    nc = bacc.Bacc()
    hist_in = nc.declare_dram_parameter("hist", [128, NOCT * 128], BF16,
                                        isOutput=False)
    qrhs_in = nc.declare_dram_parameter("qrhs", [12, max(NQ, 1)], BF16,
                                        isOutput=False)
    lhs_in = nc.declare_dram_parameter("lhsT12", [12, 128], BF16,
                                       isOutput=False)
    out_dram = nc.declare_dram_parameter("out", [128, NOCT * 128], F32,
                                         isOutput=True)

    with TileContext(nc) as tc:
        with tc.tile_pool(name="const", bufs=1) as cpool, \
             tc.tile_pool(name="work", bufs=2) as wpool, \
             tc.tile_pool(name="persist", bufs=1) as ppool, \
             tc.tile_pool(name="pspar", bufs=1, space="PSUM") as pspar, \
             tc.tile_pool(name="psq", bufs=2, space="PSUM") as psq:

            # --- inputs (first quad subtile first so the PE starts early) ---
            q0n = max(min(plan[0]["B"], QSUB) * 128, 128) if NQ else 128
            qrhs = cpool.tile([12, max(NQ, 1)], BF16)
            nc.sync.dma_start(out=qrhs[:, 0:q0n], in_=qrhs_in[:, 0:q0n])
            lhsT12 = cpool.tile([12, 128], BF16)
            nc.sync.dma_start(out=lhsT12[:], in_=lhs_in[:])
            if NQ > q0n:
                nc.sync.dma_start(out=qrhs[:, q0n:], in_=qrhs_in[:, q0n:])
            hist = cpool.tile([128, NOCT * 128], BF16)
            nc.sync.dma_start(out=hist[:, 0:512], in_=hist_in[:, 0:512])
            nc.sync.dma_start(out=hist[:, 512:1024], in_=hist_in[:, 512:1024])

            # --- setup: sigmoid table warm, U triangular ---
            warm = cpool.tile([128, 1], F32)
            nc.vector.memset(warm[:], 0.0)
            nc.scalar.activation(warm[:], warm[:], AF.Sigmoid, bias=0.0,
                                 scale=1.0)
            ui = cpool.tile([128, 128], I32)
            nc.gpsimd.iota(ui[:], pattern=[[1, 128]], base=0,
                           channel_multiplier=-1)
            ub = cpool.tile([128, 128], BF16)
            nc.vector.tensor_scalar(out=ub[:], in0=ui[:], scalar1=0,
                                    scalar2=None, op0=OP.is_ge)

            # --- parity PSUM (filled lazily, right before each sd2 group) ---
            par = pspar.tile([128, NOCT * 128], F32)

            # --- per-phase candidate pipeline ---
            d2 = ppool.tile([128, NOCT * 128], BF16)
            sd = ppool.tile([128, NOCT * 128], BF16)   # bf16 keeps TT at 2x
            val = ppool.tile([128, NOCT * 128], F32)

            # one memset covers the trailing empty phases
            kempty = NOCT
            while kempty > 0 and plan[kempty - 1]["T"] == 0:
                kempty -= 1
            if kempty < NOCT:
                nc.vector.memset(d2[:, kempty * 128:], 1000.0)

            def sd2_group(k):
                """parity matmul + sd2 + sigmoid + out DMA for phases k-1, k."""
                g0 = (k - 1) * 128
                nc.tensor.matmul(par[:, g0:g0 + 256], lhsT=ub[:],
                                 rhs=hist[:, g0:g0 + 256],
                                 start=True, stop=True)
                # par is bf16-exact (+-0.5): ACT-copy it out of PSUM so the
                # sd2 multiply runs all-bf16 at DVE 2x off the PSUM path
                parb = wpool.tile([128, 256], BF16, tag="parb")
                nc.scalar.activation(parb[:], par[:, g0:g0 + 256], AF.Copy,
                                     bias=0.0, scale=1.0)
                nc.vector.tensor_tensor(
                    out=sd[:, g0:g0 + 256], in0=parb[:],
                    in1=d2[:, g0:g0 + 256], op=OP.mult)
                nc.scalar.activation(val[:, g0:g0 + 256], sd[:, g0:g0 + 256],
                                     AF.Sigmoid, bias=0.0, scale=2.0)
                nc.sync.dma_start(out=out_dram[:, g0:g0 + 256],
                                  in_=val[:, g0:g0 + 256])

            qcol = 0
            for k in range(NOCT):
                p = plan[k]
                S, V, T, W, B = p["S"], p["V"], p["T"], p["W"], p["B"]

                if T == 0:
                    if k < kempty:
                        nc.vector.memset(d2[:, k * 128:(k + 1) * 128], 1000.0)
                    continue

                # work tile: [cand: c2 (S), verts (V) | tree scratch]
                assert W <= QSUB
                htree = (T + 1) // 2 if T > 1 else 0
                wk = wpool.tile([128, (T + htree) * 128], BF16, tag="wk")
                cand = wk[:, 0:T * 128]
                tscr = wk[:, T * 128:]

                # quads in PSUM subtiles; the w region (subtile 0 head) stays
                # in PSUM for the STT fold; c2/vert blocks copy to wk pieces
                # (alternating copy engine to balance ACT/DVE)
                nsub = (B + QSUB - 1) // QSUB
                q0t = None
                for si in range(nsub):
                    b0 = si * QSUB
                    bn = min(QSUB, B - b0)
                    q = psq.tile([128, bn * 128], F32, tag="q")
                    if si == 0:
                        q0t = q
                    for c0 in range(0, bn * 128, 512):
                        c1 = min(c0 + 512, bn * 128)
                        nc.tensor.matmul(
                            q[:, c0:c1], lhsT=lhsT12[:],
                            rhs=qrhs[:, qcol + b0 * 128 + c0:qcol + b0 * 128 + c1],
                            start=True, stop=True)
                    lo = W if si == 0 else 0        # skip w blocks
                    if bn - lo > 0:
                        dst = wk[:, (b0 + lo - W) * 128:(b0 + bn - W) * 128]
                        src = q[:, lo * 128:bn * 128]
                        if si % 2 == 0:
                            nc.scalar.activation(dst, src, AF.Copy, bias=0.0,
                                                 scale=1.0)
                        else:
                            nc.vector.tensor_scalar(out=dst, in0=src,
                                                    scalar1=0.0, scalar2=None,
                                                    op0=OP.add)
                qcol += B * 128

                # fold overshoot tests: cand c2[0:W] = max(w - 0, c2)
                if W > 0:
                    nc.vector.scalar_tensor_tensor(
                        out=cand[:, 0:W * 128], in0=q0t[:, 0:W * 128],
                        scalar=0.0, in1=cand[:, 0:W * 128],
                        op0=OP.subtract, op1=OP.max)

                # block-halving bf16 min tree -> d2 slice; ping-pong between
                # the w-stage region (dead after the fold) and the cand prefix
                d2s = d2[:, k * 128:(k + 1) * 128]
                if T == 1:
                    nc.vector.tensor_copy(out=d2s, in_=cand[:, 0:128])
                tcur = T
                src = cand
                pp = 0
                while tcur > 1:
                    half = tcur // 2
                    rem = tcur - half          # = half or half+1
                    if rem == 1:
                        dst = d2[:, k * 128:(k + 1) * 128]
                    else:
                        dst = tscr[:, 0:rem * 128] if pp == 0 \
                            else cand[:, 0:rem * 128]
                        pp ^= 1
                    nc.vector.tensor_tensor(out=dst[:, 0:half * 128],
                                            in0=src[:, 0:half * 128],
                                            in1=src[:, half * 128:2 * half * 128],
                                            op=OP.min)
                    if rem > half:   # odd leftover block passes through
                        nc.vector.tensor_copy(
                            out=dst[:, half * 128:(half + 1) * 128],
                            in_=src[:, 2 * half * 128:(2 * half + 1) * 128])
                    src = dst
                    tcur = rem

                # sd2 & sigmoid per 2-phase group (parity matmul just-in-time)
                if k % 2 == 1:
                    sd2_group(k)

    nc.finalize()
    return nc


# ---------------------------------------------------------------------------
# entry point
# ---------------------------------------------------------------------------

def kernel(polygon):
    global LAST_RESULTS
    from concourse.bass_utils import run_bass_kernel_spmd

    in_maps, core_octs, plan, NQ, row_in, col_in = _host_prep(polygon)
    nc = _build_program(plan, NQ)
    trace = bool(int(os.environ.get("KERNEL_TRACE", "0")))
    res = run_bass_kernel_spmd(nc, in_maps, list(range(NCORES)), trace=trace)
    LAST_RESULTS = res

    full = np.zeros((H, W), dtype=np.float32)
    for c in range(NCORES):
        o = res.results[c]["out"]
        for k in range(NOCT):
            s, oq = core_octs[c][k]
            full[oq * 128:(oq + 1) * 128, s * 128:(s + 1) * 128] = \
                o[:, k * 128:(k + 1) * 128]
    full[~row_in, :] = 0.0
    full[:, ~col_in] = 0.0
    return full
